# revision 6
# baseline (speedup 1.0000x reference)
"""GraphSAGE (2-layer, mean aggregation) + classifier on 8 Trainium2 NeuronCores.

Sharding: nodes split 6250/core (padded to 6272 = 49 tiles of 128).
Edges partitioned by destination core, sorted by (dst tile, src half-table),
padded to static per-tile chunk counts (max across cores -> one SPMD NEFF).

Per layer, per destination tile of 128 nodes:
  - dma_gather pulls the tile's edge source rows (256B each) from DRAM into
    SBUF, 128 edges per chunk (edge = partition).
  - a one-hot [128 edge, 128 dst] matrix is built on DVE from an iota tile via
    tensor_scalar(is_equal, mult) with per-edge dst-slot and 1/deg scalars
    (mean division folded into the one-hot).
  - PE matmul-accumulates msg_chunk.T @ onehot into PSUM -> meanT [feat, 128].
  - PE applies Wl/Wr (feature-major), ScalarE does relu+bias.
Layer-1 activations are exchanged as bf16 via two chunked AllGathers whose
outputs (24576/25600 rows, < 32768) serve directly as the two int16-indexed
gather tables for layer 2.  Classifier x_text matmuls are emitted after the
collectives so they fill the AllGather wait.
"""
import numpy as np
import concourse.bacc as bacc
import concourse.mybir as mybir
import concourse.tile as tile
from concourse.bass_utils import run_bass_kernel_spmd

F32 = mybir.dt.float32
BF16 = mybir.dt.bfloat16
I16 = mybir.dt.int16

N = 50000
NC = 8
NPC = N // NC          # 6250
T = 49                 # tiles per core
NPAD = T * 128         # 6272
NG = 7                 # gather groups
GTS = 7                # tiles per group
HALF = 32768
GD, HID, TEXT, NCLS = 64, 128, 768, 14
SPLIT = 3072           # L1-half of local nodes contributed to AllGather #1
CW = 1024              # consts width

# AllGather output tables for layer 2
T2A_ROWS = NC * SPLIT          # 24576
T2B_ROWS = NC * (NPAD - SPLIT)  # 25600


def _set_scale(n, t, ng, gts, split, half=HALF):
    """Debug helper: shrink the graph-partitioning constants (model dims
    unchanged). n must be divisible by 8; t = tiles/core = ng*gts."""
    global N, NPC, T, NPAD, NG, GTS, HALF, SPLIT, T2A_ROWS, T2B_ROWS
    N, T, NG, GTS, SPLIT, HALF = n, t, ng, gts, split, half
    NPC = N // NC
    NPAD = T * 128
    assert T == NG * GTS and NPC <= NPAD and 0 < SPLIT < NPAD
    T2A_ROWS = NC * SPLIT
    T2B_ROWS = NC * (NPAD - SPLIT)

TRACE = False
TRACE_CORES = None
LAST_RESULT = None
DEBUG_STAGE = 4  # 1=L1, 2=+AllGather, 3=+classifier, 35=full w/o collectives, 4=full


def _prep_layer(per_core):
    ngroups, gts = NG, GTS
    """per_core: list over cores of (row, tl, dl, sc) int64/float arrays.
    row = gather-table row if < HALFROWS -> table A else table B (row-HALF_OFF)
    encoded as: row already split via companion 'hf' array and rebased.
    Here we receive (rowA_or_B_rebased, hf, tl, dl, sc).
    Returns static C [T,2], idx array [128,S] int16 per core, meta [128,2*TOT]
    f32 per core, plus group offsets."""
    counts = np.zeros((NC, T, 2), np.int64)
    sorted_cores = []
    for c in range(NC):
        row, hf, tl, dl, sc = per_core[c]
        order = np.lexsort((hf, tl))
        row, hf, tl, dl, sc = (a[order] for a in (row, hf, tl, dl, sc))
        np.add.at(counts[c], (tl, hf), 1)
        sorted_cores.append((row, hf, tl, dl, sc))
    C = (counts.max(axis=0) + 127) // 128          # [T,2]
    TOT = int(C.sum())

    block_len = {}
    off = {}
    acc = 0
    for g in range(ngroups):
        for h in (0, 1):
            L = int(sum(C[t, h] for t in range(g * gts, (g + 1) * gts))) * 128
            block_len[(g, h)] = L
            off[(g, h)] = acc
            acc += L // 16
    S = max(acc, 1)

    idx_all, meta_all = [], []
    for c in range(NC):
        row, hf, tl, dl, sc = sorted_cores[c]
        key = tl * 2 + hf
        starts = np.searchsorted(key, np.arange(T * 2))
        ends = np.searchsorted(key, np.arange(T * 2), side="right")
        idx2d = np.zeros((16, S), np.int16)
        meta = np.zeros((128, 2 * max(TOT, 1)), np.float32)
        meta[:, :TOT] = -1.0
        slot = 0
        for g in range(ngroups):
            for h in (0, 1):
                col = off[(g, h)]
                for t in range(g * gts, (g + 1) * gts):
                    s0, e0 = starts[t * 2 + h], ends[t * 2 + h]
                    n = e0 - s0
                    cap = int(C[t, h]) * 128
                    if cap == 0:
                        continue
                    seg = np.zeros(cap, np.int64)
                    seg[:n] = row[s0:e0]
                    w = seg.astype(np.int16).reshape(cap // 16, 16).T
                    idx2d[:, col:col + cap // 16] = w
                    col += cap // 16
        for g in range(ngroups):
            for t in range(g * gts, (g + 1) * gts):
                for h in (0, 1):
                    s0, e0 = starts[t * 2 + h], ends[t * 2 + h]
                    n = e0 - s0
                    nch = int(C[t, h])
                    if nch == 0:
                        continue
                    cap = nch * 128
                    dseg = np.full(cap, -1.0, np.float32)
                    dseg[:n] = dl[s0:e0]
                    sseg = np.zeros(cap, np.float32)
                    sseg[:n] = sc[s0:e0]
                    meta[:, slot:slot + nch] = dseg.reshape(nch, 128).T
                    meta[:, TOT + slot:TOT + slot + nch] = \
                        sseg.reshape(nch, 128).T
                    slot += nch
        idx_all.append(np.ascontiguousarray(np.tile(idx2d, (8, 1))))
        meta_all.append(meta)
    return C, TOT, S, off, idx_all, meta_all


def _build_nc(C1, TOT1, S1, off1, C2, TOT2, S2, off2):
    nc = bacc.Bacc("TRN2", target_bir_lowering=False, debug=False,
                   num_devices=NC)
    xg_d = nc.dram_tensor("xg", [N, GD], F32, kind="ExternalInput")
    xgt_d = nc.dram_tensor("xgt", [GD, NPAD], F32, kind="ExternalInput")
    xtt_d = nc.dram_tensor("xtt", [TEXT, NPAD], F32, kind="ExternalInput")
    cst_d = nc.dram_tensor("consts", [128, CW], F32, kind="ExternalInput")
    m1_d = nc.dram_tensor("meta1", [128, 2 * max(TOT1, 1)], F32,
                          kind="ExternalInput")
    m2_d = nc.dram_tensor("meta2", [128, 2 * max(TOT2, 1)], F32,
                          kind="ExternalInput")
    i1_d = nc.dram_tensor("idx1", [128, S1], I16, kind="ExternalInput")
    i2_d = nc.dram_tensor("idx2", [128, S2], I16, kind="ExternalInput")
    lg_d = nc.dram_tensor("logits", [NPAD, NCLS], F32, kind="ExternalOutput")
    h_d = nc.dram_tensor("h", [NPAD, HID], F32, kind="ExternalOutput")

    relu = mybir.ActivationFunctionType.Relu
    iseq = mybir.AluOpType.is_equal
    mult = mybir.AluOpType.mult

    with tile.TileContext(nc) as tc:
        with (
            tc.tile_pool(name="res", bufs=1) as res,
            tc.tile_pool(name="msgp", bufs=2) as msgp,
            tc.tile_pool(name="ohp", bufs=4) as ohp,
            tc.tile_pool(name="sm", bufs=3) as sm,
            tc.tile_pool(name="ps", bufs=2, space="PSUM") as ps,
            tc.tile_pool(name="dr", bufs=1, space="DRAM") as dr,
        ):
            cst = res.tile([128, CW], F32)
            nc.sync.dma_start(cst[:], cst_d[:])
            meta1 = res.tile([128, 2 * max(TOT1, 1)], F32)
            nc.sync.dma_start(meta1[:], m1_d[:])
            meta2 = res.tile([128, 2 * max(TOT2, 1)], F32)
            nc.sync.dma_start(meta2[:], m2_d[:])
            xgt = res.tile([64, NPAD], F32)
            nc.sync.dma_start(xgt[:], xgt_d[:])
            h1T = res.tile([128, NPAD], F32)
            lgp = res.tile([128, T * NCLS], F32)

            h1locA = dr.tile([SPLIT, HID], BF16)
            h1locB = dr.tile([NPAD - SPLIT, HID], BF16)
            h1fullA = dr.tile([T2A_ROWS, HID], BF16, addr_space="Shared")
            h1fullB = dr.tile([T2B_ROWS, HID], BF16, addr_space="Shared")

            iota = cst[:, 0:128]
            ident = cst[:, 128:256]
            Wl1T = cst[0:64, 256:384]
            Wr1T = cst[0:64, 384:512]
            Wl2T = cst[:, 512:640]
            Wr2T = cst[:, 640:768]
            WcTk = [cst[:, 768 + 14 * k:768 + 14 * (k + 1)] for k in range(7)]
            bl1 = cst[:, 866:867]
            bl2 = cst[:, 867:868]
            bc_row = cst[0:1, 868:882]
            ones_row = cst[0:1, 896:1024]

            # ---------------- layer 1 ----------------
            slot = 0
            for g in range(NG):
                ts = list(range(g * GTS, (g + 1) * GTS))
                CA = [int(C1[t, 0]) for t in ts]
                CB = [int(C1[t, 1]) for t in ts]
                SCA, SCB = sum(CA), sum(CB)
                GC = SCA + SCB
                msg = msgp.tile([128, max(GC, 1), GD], F32, tag="msg")
                if SCA:
                    ia = sm.tile([128, SCA * 8], I16, tag="ia")
                    nc.sync.dma_start(
                        ia[:], i1_d[:, off1[(g, 0)]:off1[(g, 0)] + SCA * 8])
                    nc.gpsimd.dma_gather(
                        out_ap=msg[:, 0:SCA, :], in_ap=xg_d[0:HALF, :],
                        idxs_ap=ia[:], num_idxs=SCA * 128,
                        num_idxs_reg=SCA * 128, elem_size=GD,
                        single_packet=False)
                if SCB:
                    ib = sm.tile([128, SCB * 8], I16, tag="ib")
                    nc.sync.dma_start(
                        ib[:], i1_d[:, off1[(g, 1)]:off1[(g, 1)] + SCB * 8])
                    nc.gpsimd.dma_gather(
                        out_ap=msg[:, SCA:GC, :], in_ap=xg_d[HALF:N, :],
                        idxs_ap=ib[:], num_idxs=SCB * 128,
                        num_idxs_reg=SCB * 128, elem_size=GD,
                        single_packet=False)
                cumA, cumB = 0, 0
                for ti, t in enumerate(ts):
                    nch = CA[ti] + CB[ti]
                    cols = [cumA + j for j in range(CA[ti])] + \
                           [SCA + cumB + j for j in range(CB[ti])]
                    if nch:
                        pa = ps.tile([64, 128], F32, tag="agg")
                        for j, col in enumerate(cols):
                            oh = ohp.tile([128, 128], F32, tag="oh")
                            nc.vector.tensor_scalar(
                                oh[:], iota,
                                meta1[:, slot + j:slot + j + 1],
                                meta1[:, TOT1 + slot + j:TOT1 + slot + j + 1],
                                iseq, mult)
                            nc.tensor.matmul(pa[:], msg[:, col, :], oh[:],
                                             start=(j == 0),
                                             stop=(j == nch - 1))
                        mean = sm.tile([64, 128], F32, tag="mean")
                        nc.vector.tensor_copy(mean[:], pa[:])
                    slot += nch
                    ph = ps.tile([128, 128], F32, tag="h")
                    if nch:
                        nc.tensor.matmul(ph[:], Wl1T, mean[:],
                                         start=True, stop=False)
                        nc.tensor.matmul(
                            ph[:], Wr1T, xgt[:, t * 128:(t + 1) * 128],
                            start=False, stop=True)
                    else:
                        nc.tensor.matmul(
                            ph[:], Wr1T, xgt[:, t * 128:(t + 1) * 128],
                            start=True, stop=True)
                    hs = h1T[:, t * 128:(t + 1) * 128]
                    nc.scalar.activation(hs, ph[:], relu, bias=bl1)
                    ptr = ps.tile([128, 128], F32, tag="tr")
                    nc.tensor.transpose(ptr[:], hs, ident)
                    hnm = sm.tile([128, 128], BF16, tag="hnm")
                    nc.scalar.copy(hnm[:], ptr[:])
                    if t * 128 < SPLIT:
                        nc.sync.dma_start(
                            h1locA[t * 128:(t + 1) * 128, :], hnm[:])
                    else:
                        r0 = t * 128 - SPLIT
                        nc.sync.dma_start(h1locB[r0:r0 + 128, :], hnm[:])
                    cumA += CA[ti]
                    cumB += CB[ti]
                    if t == SPLIT // 128 - 1 and DEBUG_STAGE not in (1, 35):
                        nc.gpsimd.collective_compute(
                            "AllGather", mybir.AluOpType.bypass,
                            replica_groups=[list(range(NC))],
                            ins=[h1locA[:]], outs=[h1fullA[:]])
            if DEBUG_STAGE not in (1, 35):
                nc.gpsimd.collective_compute(
                    "AllGather", mybir.AluOpType.bypass,
                    replica_groups=[list(range(NC))],
                    ins=[h1locB[:]], outs=[h1fullB[:]])
            if DEBUG_STAGE == 35:
                nc.sync.dma_start(h1fullA[0:SPLIT, :], h1locA[:])
                nc.sync.dma_start(h1fullB[0:NPAD - SPLIT, :], h1locB[:])

            # ------------- classifier, x_text part (overlaps AllGather) ----
            xttv = xtt_d[:, :].rearrange("(k p) n -> p k n", p=128)
            for t in range(T if DEBUG_STAGE >= 3 else 0):
                xt = sm.tile([128, 6, 128], F32, tag="xt")
                nc.sync.dma_start(xt[:], xttv[:, :, t * 128:(t + 1) * 128])
                pl = ps.tile([128, NCLS], F32, tag="lg")
                for k in range(6):
                    nc.tensor.matmul(pl[:], xt[:, k, :], WcTk[k],
                                     start=(k == 0), stop=False)
                nc.tensor.matmul(pl[:], ones_row, bc_row,
                                 start=False, stop=True)
                nc.vector.tensor_copy(lgp[:, t * NCLS:(t + 1) * NCLS], pl[:])

            # ---------------- layer 2 ----------------
            slot = 0
            for g in range(NG if DEBUG_STAGE in (35, 4) else 0):
                ts = list(range(g * GTS, (g + 1) * GTS))
                CA = [int(C2[t, 0]) for t in ts]
                CB = [int(C2[t, 1]) for t in ts]
                SCA, SCB = sum(CA), sum(CB)
                GC = SCA + SCB
                msg = msgp.tile([128, max(GC, 1), HID], BF16, tag="msg")
                if SCA:
                    ia = sm.tile([128, SCA * 8], I16, tag="ia")
                    nc.sync.dma_start(
                        ia[:], i2_d[:, off2[(g, 0)]:off2[(g, 0)] + SCA * 8])
                    nc.gpsimd.dma_gather(
                        out_ap=msg[:, 0:SCA, :], in_ap=h1fullA[:],
                        idxs_ap=ia[:], num_idxs=SCA * 128,
                        num_idxs_reg=SCA * 128, elem_size=HID,
                        single_packet=False)
                if SCB:
                    ib = sm.tile([128, SCB * 8], I16, tag="ib")
                    nc.sync.dma_start(
                        ib[:], i2_d[:, off2[(g, 1)]:off2[(g, 1)] + SCB * 8])
                    nc.gpsimd.dma_gather(
                        out_ap=msg[:, SCA:GC, :], in_ap=h1fullB[:],
                        idxs_ap=ib[:], num_idxs=SCB * 128,
                        num_idxs_reg=SCB * 128, elem_size=HID,
                        single_packet=False)
                cumA, cumB = 0, 0
                for ti, t in enumerate(ts):
                    nch = CA[ti] + CB[ti]
                    cols = [cumA + j for j in range(CA[ti])] + \
                           [SCA + cumB + j for j in range(CB[ti])]
                    if nch:
                        pa = ps.tile([128, 128], F32, tag="agg")
                        for j, col in enumerate(cols):
                            oh = ohp.tile([128, 128], BF16, tag="oh")
                            nc.vector.tensor_scalar(
                                oh[:], iota,
                                meta2[:, slot + j:slot + j + 1],
                                meta2[:, TOT2 + slot + j:TOT2 + slot + j + 1],
                                iseq, mult)
                            nc.tensor.matmul(pa[:], msg[:, col, :], oh[:],
                                             start=(j == 0),
                                             stop=(j == nch - 1))
                        mean = sm.tile([128, 128], F32, tag="mean")
                        nc.vector.tensor_copy(mean[:], pa[:])
                    slot += nch
                    ph = ps.tile([128, 128], F32, tag="h")
                    if nch:
                        nc.tensor.matmul(ph[:], Wl2T, mean[:],
                                         start=True, stop=False)
                        nc.tensor.matmul(
                            ph[:], Wr2T, h1T[:, t * 128:(t + 1) * 128],
                            start=False, stop=True)
                    else:
                        nc.tensor.matmul(
                            ph[:], Wr2T, h1T[:, t * 128:(t + 1) * 128],
                            start=True, stop=True)
                    h2 = sm.tile([128, 128], F32, tag="h2")
                    nc.scalar.activation(h2[:], ph[:], relu, bias=bl2)
                    pl2 = ps.tile([128, NCLS], F32, tag="lg")
                    nc.tensor.matmul(pl2[:], h2[:], WcTk[6],
                                     start=True, stop=True)
                    lgt = sm.tile([128, NCLS], F32, tag="lgt")
                    nc.vector.tensor_tensor(
                        lgt[:], pl2[:], lgp[:, t * NCLS:(t + 1) * NCLS],
                        mybir.AluOpType.add)
                    nc.sync.dma_start(lg_d[t * 128:(t + 1) * 128, :], lgt[:])
                    ptr = ps.tile([128, 128], F32, tag="tr")
                    nc.tensor.transpose(ptr[:], h2[:], ident)
                    h2nm = sm.tile([128, 128], F32, tag="hnm")
                    nc.scalar.copy(h2nm[:], ptr[:])
                    nc.sync.dma_start(h_d[t * 128:(t + 1) * 128, :], h2nm[:])
                    cumA += CA[ti]
                    cumB += CB[ti]
            if DEBUG_STAGE in (1, 2, 3):
                zl = sm.tile([128, NCLS], F32, tag="lgt")
                nc.vector.memset(zl[:], 0.0)
                zh = sm.tile([128, HID], F32, tag="hnm")
                nc.vector.memset(zh[:], 0.0)
                for t in range(T):
                    nc.sync.dma_start(lg_d[t * 128:(t + 1) * 128, :], zl[:])
                    nc.sync.dma_start(h_d[t * 128:(t + 1) * 128, :], zh[:])
    nc.compile()
    return nc


def kernel(x_text, x_graph, edge_index, Wl1, bl1, Wr1, Wl2, bl2, Wr2, Wc, bc):
    global LAST_RESULT
    x_text = np.asarray(x_text, np.float32)
    x_graph = np.asarray(x_graph, np.float32)
    edge_index = np.asarray(edge_index, np.int64)
    Wl1, bl1, Wr1 = (np.asarray(a, np.float32) for a in (Wl1, bl1, Wr1))
    Wl2, bl2, Wr2 = (np.asarray(a, np.float32) for a in (Wl2, bl2, Wr2))
    Wc, bc = np.asarray(Wc, np.float32), np.asarray(bc, np.float32)

    src, dst = edge_index[0], edge_index[1]
    deg = np.bincount(dst, minlength=N).astype(np.float32)
    invd = (1.0 / np.maximum(deg, 1.0)).astype(np.float32)

    core = dst // NPC
    per1, per2 = [], []
    for c in range(NC):
        m = core == c
        s, d = src[m], dst[m] - c * NPC
        tl = d // 128
        dl = (d % 128).astype(np.float32)
        sc = invd[dst[m]]
        # layer-1 tables: x_graph rows split at HALF
        hf1 = (s >= HALF).astype(np.int64)
        row1 = s - HALF * hf1
        per1.append((row1, hf1, tl, dl, sc))
        # layer-2 tables: AllGather outputs A (local<SPLIT) / B
        sl = s % NPC
        sco = s // NPC
        hf2 = (sl >= SPLIT).astype(np.int64)
        row2 = np.where(hf2 == 0, sco * SPLIT + sl,
                        sco * (NPAD - SPLIT) + (sl - SPLIT))
        per2.append((row2, hf2, tl, dl, sc))

    C1, TOT1, S1, off1, idx1_all, meta1_all = _prep_layer(per1)
    C2, TOT2, S2, off2, idx2_all, meta2_all = _prep_layer(per2)

    # consts block
    WcT = Wc.T  # [896, 14]
    cst = np.zeros((128, CW), np.float32)
    cst[:, 0:128] = np.arange(128, dtype=np.float32)[None, :]
    cst[:, 128:256] = np.eye(128, dtype=np.float32)
    cst[0:64, 256:384] = Wl1.T            # [64,128]
    cst[0:64, 384:512] = Wr1.T
    cst[:, 512:640] = Wl2.T
    cst[:, 640:768] = Wr2.T
    cst[:, 768:866] = np.transpose(
        WcT.reshape(7, 128, NCLS), (1, 0, 2)).reshape(128, 7 * NCLS)
    cst[:, 866] = bl1
    cst[:, 867] = bl2
    cst[0, 868:882] = bc
    cst[0, 896:1024] = 1.0

    in_maps = []
    for c in range(NC):
        xgt = np.zeros((GD, NPAD), np.float32)
        xgt[:, :NPC] = x_graph[c * NPC:(c + 1) * NPC].T
        xtt = np.zeros((TEXT, NPAD), np.float32)
        xtt[:, :NPC] = x_text[c * NPC:(c + 1) * NPC].T
        in_maps.append({
            "xg": x_graph,
            "xgt": xgt,
            "xtt": np.ascontiguousarray(xtt),
            "consts": cst,
            "meta1": meta1_all[c],
            "meta2": meta2_all[c],
            "idx1": idx1_all[c],
            "idx2": idx2_all[c],
        })

    nc = _build_nc(C1, TOT1, S1, off1, C2, TOT2, S2, off2)
    r = run_bass_kernel_spmd(
        nc, in_maps, core_ids=list(range(NC)),
        trace=TRACE, trace_cores=TRACE_CORES)
    LAST_RESULT = r

    logits = np.concatenate(
        [r.results[c]["logits"][:NPC] for c in range(NC)], axis=0)
    h = np.concatenate(
        [r.results[c]["h"][:NPC] for c in range(NC)], axis=0)
    return logits.astype(np.float32), h.astype(np.float32)
